# revision 32
# baseline (speedup 1.0000x reference)
"""GQA causal-attention prefill kernel for 8 TRN2 NeuronCores.

Sharding (zero cross-core comm): 8 KV heads -> 1 per core, each with its 4
GQA query heads. Per core: Wq slice [512,2560], Wk/Wv slice [128,2560], Wo
column-slice [2560,512]. Each core computes a full [2048,2560] partial of
the output projection; the host sums the 8 partials.

Per-core math (all matmuls in float32r: fp32 data at 1 cycle/row):
  proj:  q/k/v seq-major via stationary=hsT tiles, moving=W^T slabs
  norm+rope on seq-major tiles (DVE), fold norm weights into cos/sin tables
  PE-transpose roped q,k -> feature-major qT,kT
  S^T = kT_tile.T @ qT  (only lower-triangle blocks; diag blocks masked)
  P^T = exp(scale*S^T)  (ACT, no max-subtraction: |scale*S| ~ <6)
  rowsum via ones-vector matmul; AV: attnT = v.T @ P^T (born feature-major)
  normalize at AV evict via K=1 ones broadcast matmul + approx reciprocal
  out_partial = attnT.T @ WoT accumulated over 4 head-feat tiles

All heavy DMAs use host-pre-tiled contiguous layouts (128 x big-row
descriptors instead of tens of thousands of 512B ones).
"""

import ml_dtypes
import numpy as np

import concourse.bass as bass
import concourse.mybir as mybir
import concourse.tile as tile
from concourse import bacc
from concourse.bass_utils import run_bass_kernel_spmd

P = 128
S = 2048
H = 2560
NS = S // P          # 16 s-tiles
NHT = H // P         # 20 hidden tiles
NH = 4               # q heads per core
DQ = NH * P          # 512
DKV = 2 * P          # 256 (k|v)
NCH = 4              # sq chunks of 512
CW = 512
NJC = H // CW        # 5 output col chunks
SCALE = float(P) ** -0.5
EPS = 1e-6

F32 = mybir.dt.float32
F32R = mybir.dt.float32r
BF16 = mybir.dt.bfloat16

_CACHE = {}


def _build():
    nc = bacc.Bacc("TRN2", target_bir_lowering=False)

    hst = nc.declare_dram_parameter("hst", [NS, P, NHT * P], F32R, isOutput=False)
    wqt = nc.declare_dram_parameter("wqt", [P, NHT * DQ], F32R, isOutput=False)
    wkvt = nc.declare_dram_parameter("wkvt", [P, NHT * DKV], F32R, isOutput=False)
    wot = nc.declare_dram_parameter("wot", [P, NH * H], F32R, isOutput=False)
    tab = nc.declare_dram_parameter("tab", [NS, P, 4 * P], F32, isOutput=False)
    cmask = nc.declare_dram_parameter("cmask", [P, P], F32, isOutput=False)
    ident = nc.declare_dram_parameter("ident", [P, P], F32R, isOutput=False)
    onessq = nc.declare_dram_parameter("onessq", [P, P], F32R, isOutput=False)
    out = nc.declare_dram_parameter("out", [S, H], F32, isOutput=True)

    with tile.TileContext(nc) as tc:
        with (
            tc.tile_pool(name="wq", bufs=1) as wq_pool,        # WqT slab -> WoT slab
            tc.tile_pool(name="wkv", bufs=1) as wkv_pool,      # WkvT slab
            tc.tile_pool(name="qa", bufs=6) as qa_pool,        # qT_g then attnT_g [128,2048]
            tc.tile_pool(name="kv", bufs=1) as kv_pool,        # kT + v
            tc.tile_pool(name="big", bufs=3) as big_pool,      # hsT slabs / out staging
            tc.tile_pool(name="tab", bufs=3) as tab_pool,      # streamed table blocks
            tc.tile_pool(name="wk", bufs=9) as wk_pool,        # rope scratch + recipb
            tc.tile_pool(name="qw", bufs=4) as qw_pool,        # roped q/k (f32r)
            tc.tile_pool(name="pt", bufs=8) as pt_pool,        # P^T tiles
            tc.tile_pool(name="sm", bufs=4) as sm_pool,        # small stats
            tc.tile_pool(name="cst", bufs=1) as cst_pool,      # consts
            tc.tile_pool(name="ps", bufs=7, space="PSUM") as ps_pool,
            tc.tile_pool(name="tp", bufs=1, space="PSUM") as tp_pool,
        ):
            slabs, tabts = {}, {}

            def load_inputs(i):
                if i in slabs or i >= NS:
                    return
                slabs[i] = big_pool.tile([P, NHT * P], F32R, tag="big",
                                         name=f"slab_{i}")
                nc.sync.dma_start(slabs[i][:], hst.ap()[i])
                tabts[i] = tab_pool.tile([P, 4 * P], F32, tag="tab",
                                         name=f"tabt_{i}")
                nc.sync.dma_start(tabts[i][:], tab.ap()[i])

            wqt_sb = wq_pool.tile([P, NHT * DQ], F32R, tag="w")
            wkvt_sb = wkv_pool.tile([P, NHT * DKV], F32R, tag="w")

            # ---- consts ----
            cmask_sb = cst_pool.tile([P, P], F32, tag="cmask")
            nc.sync.dma_start(cmask_sb[:], cmask.ap())
            ident_sb = cst_pool.tile([P, P], F32R, tag="ident")
            nc.sync.dma_start(ident_sb[:], ident.ap())
            onessq_sb = cst_pool.tile([P, P], F32R, tag="onessq")
            nc.sync.dma_start(onessq_sb[:], onessq.ap())
            eps_sb = cst_pool.tile([P, 1], F32, tag="eps")
            nc.vector.memset(eps_sb[:], EPS)



            # persistent attention operands
            kT = kv_pool.tile([P, S], F32R, tag="kt")          # [d, t]
            v_sb = kv_pool.tile([P, NS, P], F32R, tag="v")     # [t, tile, d]

            # qT_g: [d, r(s-tile within chunk), h, s] per chunk g
            qT = [None] * NCH

            # ============ interleaved: proj s-tiles + attention chunks ===========
            attnT = [None] * NCH

            def proj_tile(i):
                g, r = i // NCH, i % NCH
                load_inputs(i)
                load_inputs(i + 1)
                slab, tabt = slabs.pop(i), tabts.pop(i)
                cq_t, sq_t = tabt[:, 0:P], tabt[:, P:2 * P]
                ck_t, sk_t = tabt[:, 2 * P:3 * P], tabt[:, 3 * P:4 * P]

                q_ps = ps_pool.tile([P, DQ], F32, tag="mm")
                kv_ps = ps_pool.tile([P, DKV], F32, tag="mm")
                for t in range(NHT):
                    nc.tensor.matmul(
                        q_ps[:], slab[:, t * P:(t + 1) * P],
                        wqt_sb[:, t * DQ:(t + 1) * DQ],
                        start=(t == 0), stop=(t == NHT - 1),
                    )
                for t in range(NHT):
                    nc.tensor.matmul(
                        kv_ps[:], slab[:, t * P:(t + 1) * P],
                        wkvt_sb[:, t * DKV:(t + 1) * DKV],
                        start=(t == 0), stop=(t == NHT - 1),
                    )

                # v: plain evict (cols 128:256 of kv)
                nc.vector.tensor_copy(v_sb[:, i, :], kv_ps[:, P:DKV])

                # ---- rms-norm stats (ACT square w/ accum) ----
                q2 = wk_pool.tile([P, DQ], F32, tag="wk", name=f"q2_{i}")
                nc.scalar.activation(
                    q2[:], q_ps[:, 0:DQ],
                    mybir.ActivationFunctionType.Square,
                )
                ss = sm_pool.tile([P, NH + 1], F32, tag="ssq")
                nc.vector.tensor_reduce(
                    ss[:, 0:NH], q2[:].rearrange("p (h d) -> p h d", h=NH),
                    mybir.AxisListType.X, mybir.AluOpType.add,
                )
                junk = sm_pool.tile([P, P], F32, tag="junk")
                nc.scalar.activation(
                    junk[:], kv_ps[:, 0:P],
                    mybir.ActivationFunctionType.Square,
                    accum_out=ss[:, NH:NH + 1],
                )
                rstd = sm_pool.tile([P, NH + 1], F32, tag="rsq")
                nc.scalar.activation(
                    rstd[:], ss[:], mybir.ActivationFunctionType.Sqrt,
                    bias=eps_sb[:], scale=1.0 / P,
                )
                nc.vector.reciprocal_approx_fast(rstd[:], rstd[:])
                rstd_q, rstd_k = rstd[:, 0:NH], rstd[:, NH:NH + 1]

                # ---- fused norm-scale + rope (DVE) ----
                def rope(ps_slice, nh, rstd, cos_t, sin_t, nm):
                    w = nh * P
                    qn = wk_pool.tile([P, w], F32, tag="wk", name=f"qn_{nm}_{i}")
                    q3 = qn[:].rearrange("p (h d) -> p h d", h=nh)
                    nc.vector.tensor_tensor(
                        q3, ps_slice.rearrange("p (h d) -> p h d", h=nh),
                        rstd[:, :, None].broadcast_to([P, nh, P]),
                        mybir.AluOpType.mult,
                    )
                    r1 = wk_pool.tile([P, w], F32, tag="wk", name=f"r1_{nm}_{i}")
                    nc.vector.tensor_tensor(
                        r1[:].rearrange("p (h d) -> p h d", h=nh), q3,
                        cos_t[:, None, :].broadcast_to([P, nh, P]),
                        mybir.AluOpType.mult,
                    )
                    r2 = wk_pool.tile([P, w], F32, tag="wk", name=f"r2_{nm}_{i}")
                    r23 = r2[:].rearrange("p (h d) -> p h d", h=nh)
                    nc.vector.tensor_tensor(
                        r23[:, :, 0:64], q3[:, :, 64:P],
                        sin_t[:, None, 0:64].broadcast_to([P, nh, 64]),
                        mybir.AluOpType.mult,
                    )
                    nc.vector.tensor_tensor(
                        r23[:, :, 64:P], q3[:, :, 0:64],
                        sin_t[:, None, 64:P].broadcast_to([P, nh, 64]),
                        mybir.AluOpType.mult,
                    )
                    ro = qw_pool.tile([P, w], F32R, tag="qw", name=f"ro_{nm}_{i}")
                    nc.vector.tensor_tensor(
                        ro[:], r1[:], r2[:], mybir.AluOpType.add,
                    )
                    return ro

                q_ro = rope(q_ps[:, 0:DQ], NH, rstd_q, cq_t, sq_t, "q")
                k_ro = rope(kv_ps[:, 0:P], 1, rstd_k, ck_t, sk_t, "k")

                # ---- transpose to feature-major (4 transposes -> 1 bank) ----
                tpq = tp_pool.tile([P, DQ], F32R, tag="tp")
                for h in range(NH):
                    nc.tensor.transpose(
                        tpq[:, h * P:(h + 1) * P], q_ro[:, h * P:(h + 1) * P],
                        ident_sb[:],
                    )
                if qT[g] is None:
                    qT[g] = qa_pool.tile([P, NCH, NH, P], F32R, tag="qa",
                                         name=f"qT_{g}")
                nc.vector.tensor_copy(
                    qT[g][:, r, :, :].rearrange("p h d -> p (h d)"), tpq[:],
                )
                tpk = tp_pool.tile([P, P], F32R, tag="tp", name=f"tpk_{i}")
                nc.tensor.transpose(tpk[:], k_ro[:], ident_sb[:])
                nc.vector.tensor_copy(kT[:, i * P:(i + 1) * P], tpk[:])

            def attn_unit(g, h):
                av_ps = ps_pool.tile([P, CW], F32, tag="mm", name=f"av_{g}_{h}")
                rbacc = wk_pool.tile([P, CW], F32R, tag="wk", name=f"ra_{g}_{h}")
                njt = 4 * g + 4  # t-tiles 0..4g+3
                for j in range(njt):
                    r0 = max(0, j - 4 * g)
                    off = r0 * P
                    w = CW - off
                    st_ps = ps_pool.tile([P, CW], F32, tag="mm",
                                         name=f"st_{g}_{h}_{j}")
                    nc.tensor.matmul(
                        st_ps[:, 0:w],
                        kT[:, j * P:(j + 1) * P],
                        qT[g][:, r0:NCH, h, :],
                    )
                    if j >= 4 * g:
                        nc.vector.tensor_tensor(
                            st_ps[:, 0:P], st_ps[:, 0:P], cmask_sb[:],
                            mybir.AluOpType.add,
                        )
                    ptile = pt_pool.tile([P, CW], F32R, tag="pt",
                                         name=f"pt_{g}_{h}_{j}")
                    nc.scalar.activation(
                        ptile[:, 0:w], st_ps[:, 0:w],
                        mybir.ActivationFunctionType.Exp, scale=SCALE,
                    )
                    if j == 0:
                        nc.vector.tensor_copy(rbacc[:], ptile[:])
                    else:
                        nc.vector.tensor_tensor(
                            rbacc[:, off:off + w], rbacc[:, off:off + w],
                            ptile[:, 0:w], mybir.AluOpType.add,
                        )
                    nc.tensor.matmul(
                        av_ps[:, off:off + w], v_sb[:, j, :], ptile[:, 0:w],
                        start=(j == 0), stop=(j == njt - 1),
                    )
                # normalize: one cross-partition reduce+broadcast matmul,
                # approx reciprocal, fused evict
                rb_ps = ps_pool.tile([P, CW], F32, tag="mm", name=f"rb_{g}_{h}")
                nc.tensor.matmul(rb_ps[:], onessq_sb[:], rbacc[:])
                recipb = wk_pool.tile([P, CW], F32, tag="wk",
                                      name=f"rc_{g}_{h}")
                nc.vector.reciprocal_approx_fast(recipb[:], rb_ps[:])
                if attnT[g] is None:
                    attnT[g] = qa_pool.tile([P, NH, CW], F32R, tag="qa",
                                            name=f"attnT_{g}")
                nc.vector.tensor_tensor(
                    attnT[g][:, h, :], av_ps[:], recipb[:],
                    mybir.AluOpType.mult,
                )

            load_inputs(0)
            for cc in range(10):
                nc.sync.dma_start(
                    wqt_sb[:, cc * 2 * DQ:(cc + 1) * 2 * DQ],
                    wqt.ap()[:, cc * 2 * DQ:(cc + 1) * 2 * DQ],
                )
            nc.sync.dma_start(wkvt_sb[:], wkvt.ap())
            for i in range(NS):
                proj_tile(i)
                if i % NCH == 0 and i >= NCH:
                    for h in range(NH):
                        attn_unit(i // NCH - 1, h)

            # ====== phase 3: output projection, interleaved with attn chunk 3 ====
            wot_sb = wq_pool.tile([P, NH * H], F32R, tag="w")
            nc.sync.dma_start(wot_sb[:], wot.ap())

            def wo_tile(i):
                g, r = i // NCH, i % NCH
                o_stage = big_pool.tile([P, H], F32, tag="big", name=f"ost_{i}")
                for jc in range(NJC):
                    o_ps = ps_pool.tile([P, CW], F32, tag="mm", name=f"op_{i}_{jc}")
                    for f in range(NH):
                        nc.tensor.matmul(
                            o_ps[:],
                            attnT[g][:, f, r * P:(r + 1) * P],
                            wot_sb[:, f * H + jc * CW:f * H + (jc + 1) * CW],
                            start=(f == 0), stop=(f == NH - 1),
                        )
                    eng = nc.scalar.copy if jc % 2 == 0 else nc.vector.tensor_copy
                    eng(o_stage[:, jc * CW:(jc + 1) * CW], o_ps[:])
                nc.sync.dma_start(out.ap()[i * P:(i + 1) * P, :], o_stage[:])

            for h in range(NH):
                attn_unit(NCH - 1, h)
            for i in range(NS):
                wo_tile(i)
    nc.compile()
    return nc


def kernel(hidden_states, cos, sin, Wq, Wk, Wv, Wo, q_norm_w, k_norm_w):
    hs = np.asarray(hidden_states, dtype=np.float32)[0]      # [S, H]
    cos0 = np.asarray(cos, dtype=np.float32)[0]              # [S, 128]
    sin0 = np.asarray(sin, dtype=np.float32)[0]
    Wq = np.asarray(Wq, dtype=np.float32)
    Wk = np.asarray(Wk, dtype=np.float32)
    Wv = np.asarray(Wv, dtype=np.float32)
    Wo = np.asarray(Wo, dtype=np.float32)
    qw = np.asarray(q_norm_w, dtype=np.float32)
    kw = np.asarray(k_norm_w, dtype=np.float32)

    # slab[i][p][t*128+s] = hs[i*128+s, t*128+p]
    hst_t = np.ascontiguousarray(
        hs.reshape(NS, P, NHT, P).transpose(0, 3, 2, 1).reshape(NS, P, NHT * P)
    )
    sgn = np.concatenate([-np.ones(64, np.float32), np.ones(64, np.float32)])

    def tables(w):
        wr = np.concatenate([w[64:], w[:64]])                # w[(i+64)%128]
        return cos0 * w[None, :], sin0 * (sgn * wr)[None, :]

    cosq_t, sinq_t = tables(qw)
    cosk_t, sink_t = tables(kw)
    tab_t = np.ascontiguousarray(
        np.concatenate([cosq_t, sinq_t, cosk_t, sink_t], axis=1)
        .astype(np.float32).reshape(NS, P, 4 * P)
    )
    idx = np.arange(P)
    cmask_np = np.where(idx[None, :] >= idx[:, None], 0.0, -1e30).astype(np.float32)
    ident_np = np.eye(P, dtype=np.float32)
    onessq_np = np.ones((P, P), np.float32)

    if "nc" not in _CACHE:
        _CACHE["nc"] = _build()
    nc = _CACHE["nc"]

    in_maps = []
    for c in range(8):
        wq_c = Wq[c * DQ:(c + 1) * DQ, :]                    # [512, H]
        wqt_t = np.ascontiguousarray(
            wq_c.reshape(DQ, NHT, P).transpose(2, 1, 0).reshape(P, NHT * DQ)
        )
        kv_c = np.concatenate([Wk[c * P:(c + 1) * P, :], Wv[c * P:(c + 1) * P, :]], axis=0)
        wkvt_t = np.ascontiguousarray(
            kv_c.reshape(DKV, NHT, P).transpose(2, 1, 0).reshape(P, NHT * DKV)
        )
        wot_c = np.ascontiguousarray(Wo[:, c * DQ:(c + 1) * DQ].T)  # [512, H]
        wot_t = np.ascontiguousarray(
            wot_c.reshape(NH, P, H).transpose(1, 0, 2).reshape(P, NH * H)
        )
        in_maps.append(dict(
            hst=hst_t, wqt=wqt_t, wkvt=wkvt_t, wot=wot_t, tab=tab_t,
            cmask=cmask_np, ident=ident_np, onessq=onessq_np,
        ))

    try:
        r = run_bass_kernel_spmd(nc, in_maps, core_ids=list(range(8)))
    except Exception:
        r = run_bass_kernel_spmd(nc, in_maps, core_ids=list(range(8)))
    acc = np.zeros((S, H), dtype=np.float32)
    for c in range(8):
        acc += r.results[c]["out"]
    return acc[None, :, :]
